# revision 1
# baseline (speedup 1.0000x reference)
"""DistanceCentroidLoss on 8 Trainium2 NeuronCores (Bass/Tile).

Data-parallel over N: each core processes 32768 rows. Per-core device work:
  d2[n,j] = |x_n|^2 + |c_j|^2 - 2 x.c_j   (bf16 inputs, f32 PSUM accumulation)
  dist    = sqrt(d2)
  G[0:64,  k] = sum_n dist[n,j] * onehot[n,k]   (PE segment matmuls)
  G[64:128,k] = sum_n d2[n,j]   * onehot[n,k]
  cnt[t,k]    = sum_n onehot[n,k]
The tiny cross-core/cluster reduction to the scalar loss runs on host:
  A_k = G_d2[k,k],  V_k = colsum(G_d2)_k - A_k,  W_k = colsum(G_dist)_k - G_dist[k,k]
  loss = (1/K) sum_k [ A_k + (M^2*63*c_k - 2M*W_k + V_k)/63 ] / max(c_k,1)

Toolchain quirk: this walrus build rejects any instruction with more than one
semaphore wait. The kernel is structured so every instruction needs at most
one cross-proc wait (single SWDGE/HWDGE completion lanes, persistent arenas
instead of rotating cross-engine tiles, wait-carrying instruction ordering),
plus a conservative post-pass that strips semantically-redundant waits Tile
emits (own-stream tautologies and already-observed sem values).
"""
import sys

sys.path.insert(0, '/opt/trn_rl_repo')

import numpy as np
import ml_dtypes

import concourse.bass as bass
import concourse.mybir as mybir
import concourse.tile_sem_assignment as _tsa
from concourse.tile import TileContext, add_dep_helper
from concourse.bass_utils import run_bass_kernel_spmd

f32 = mybir.dt.float32
bf16 = mybir.dt.bfloat16
f16 = mybir.dt.float16
i32 = mybir.dt.int32

N, D, K = 262144, 128, 64
NCORES = 8
NS = N // NCORES        # rows per core = 32768
T = NS // 128           # 256 tiles of 128 rows
TPB = 8                 # tiles per block
NB = T // TPB           # 32 blocks
TPC = 32                # tiles per DMA chunk
NCH = T // TPC          # 8 chunks
BPC = TPC // TPB        # blocks per chunk = 4
MARGIN = 5.0

_cache = {}

_OWN_PREFIX = {
    "Activation": ("Activation_",),
    "DVE": ("DVE_",),
    "PE": ("PE_",),
    "Pool": ("Pool_",),
    "SP": ("SP_sequencer",),
}


def _is_own(eng, name):
    for p in _OWN_PREFIX.get(eng, ()):
        if name.startswith(p) and not name.startswith("barrier"):
            return True
    return False


def _strip_redundant_waits(nc):
    """Drop tautological sem waits (see module docstring)."""
    own, seen = {}, {}
    for blk in nc.m.functions[0].blocks:
        for inst in blk.instructions:
            eng = str(inst.engine).split(".")[-1]
            si = inst.sync_info
            oc = own.setdefault(eng, {})
            ob = seen.setdefault(eng, {})
            if si is not None and si.on_wait:
                keep = []
                for w in si.on_wait:
                    if w.sync_type != "semaphore" or w.wait_mode != "sem-ge-imm" \
                            or w.wait_reg is not None or w.wait_value is None:
                        keep.append(w)
                        continue
                    nm, v = w.ant_name, w.wait_value
                    if "barrier" in nm:
                        # barrier EVSEMs are sem-sub'd (non-monotonic): never elide
                        keep.append(w)
                        continue
                    if ob.get(nm, -1) >= v or (_is_own(eng, nm) and oc.get(nm, 0) >= v):
                        continue
                    keep.append(w)
                    ob[nm] = max(ob.get(nm, -1), v)
                if len(keep) != len(si.on_wait):
                    si.on_wait = keep
                    inst.sync_info = si
            if si is not None and si.on_update:
                dma = "DMA" in type(inst).__name__ or "DmaTranspose" in type(inst).__name__
                for u in si.on_update:
                    if u.update_mode != "sem-inc" or u.update_value is None:
                        continue
                    if not dma and _is_own(eng, u.ant_name):
                        oc[u.ant_name] = oc.get(u.ant_name, 0) + u.update_value
    return nc


def _build(repeat=1):
    # single completion-sem lane per DGE class: all SWDGE (gpsimd) DMAs share
    # DMASW0 and all HWDGE transposes share DMAHW0, so consumers never carry
    # waits on more than one DMA proc (execution is FIFO per DGE queue, so a
    # single monotone lane is sound).
    _tsa.NUM_SWDGE_GLOBAL_SEMS = 1
    _tsa.NUM_HWDGE_SEMS = 1

    nc = bass.Bass()
    x_in = nc.dram_tensor("x", [NS, D], f32, kind="ExternalInput")
    lab_in = nc.dram_tensor("lab", [128, 2 * T], i32, kind="ExternalInput")
    cTm2_in = nc.dram_tensor("cTm2", [D, K], bf16, kind="ExternalInput")
    ones64_in = nc.dram_tensor("ones64", [D, K], f16, kind="ExternalInput")
    iota_in = nc.dram_tensor("iota64", [128, K], f32, kind="ExternalInput")
    c2b_in = nc.dram_tensor("c2b8", [128, TPB * K], f32, kind="ExternalInput")
    out_G = nc.dram_tensor("out_G", [128, K], f32, kind="ExternalOutput")
    out_cnt = nc.dram_tensor("out_cnt", [1, TPB * K], f32, kind="ExternalOutput")

    x_rtd = x_in[:].rearrange("(r t) d -> r t d", t=T)  # row n = r*T + t

    with TileContext(nc) as tc:
        with tc.tile_pool(name="single", bufs=1) as sb, \
             tc.tile_pool(name="xbfp", bufs=2) as xbfp, \
             tc.tile_pool(name="xsqp", bufs=1) as xsqp, \
             tc.tile_pool(name="ohp", bufs=2) as ohp, \
             tc.tile_pool(name="dscrp", bufs=1) as dscrp, \
             tc.tile_pool(name="pxc", bufs=2, space="PSUM") as pxc, \
             tc.tile_pool(name="pacc", bufs=1, space="PSUM") as pacc:

            # ---- constants, all on the single SWDGE lane; labels LAST so the
            # first DVE wait on the lane covers the whole constant prefix ----
            cTm2_sb = sb.tile([D, K], bf16)
            nc.gpsimd.dma_start(out=cTm2_sb, in_=cTm2_in[:])
            ones64_sb = sb.tile([D, K], f16)
            nc.gpsimd.dma_start(out=ones64_sb, in_=ones64_in[:])
            iota_sb = sb.tile([128, K], f32)
            nc.gpsimd.dma_start(out=iota_sb, in_=iota_in[:])
            c2b_sb = sb.tile([128, TPB * K], f32)
            nc.gpsimd.dma_start(out=c2b_sb, in_=c2b_in[:])
            lab_sb = sb.tile([128, 2 * T], i32)
            nc.gpsimd.dma_start(out=lab_sb, in_=lab_in[:])
            labf = sb.tile([128, T], f32)
            i_labf = nc.vector.tensor_copy(out=labf, in_=lab_sb[:, 0::2])
            ones1 = sb.tile([128, 1], f16)
            nc.vector.memset(ones1, 1.0)
            scr_a = sb.tile([1, 1], f32)
            nc.vector.memset(scr_a, 0.0)
            scr_b = sb.tile([1, 1], f32)
            # initial ACT instruction observes DVE so later touches carry only
            # their PE wait
            nc.scalar.copy(out=scr_b, in_=scr_a)

            # ---- persistent arenas (separate per-chunk tiles: no WAW/WAR
            # chains between chunk DMAs) ----
            x_bfs = [sb.tile([128, TPC, 128], bf16, name=f"xbf{c}") for c in range(NCH)]
            xTs = [sb.tile([128, TPC, 128], bf16, name=f"xT{c}") for c in range(NCH)]
            dd = sb.tile([128, T, 2, K], f16)         # 8 MB: [dist | d2]

            G_ps = pacc.tile([128, K], f32)
            cnt_ps = pacc.tile([1, TPB * K], f32)

            casts, tps, end_list = [], [], []
            prev_mmB_last = None
            prev_rep = {}   # per-chunk: (last square, last mmB) of previous rep

            for rep in range(repeat):
              for c in range(NCH):
                x_bf = x_bfs[c]
                if rep > 0:
                    # cast overwrites x_bf read by last rep's transpose
                    pnr = nc.gpsimd.nop()
                    add_dep_helper(pnr.ins, prev_rep[c]["tp"].ins, sync=True,
                                   reason="rep: pool observes tp")
                if c > 0 or rep > 0:
                    # shield the cast's xpose-serialization wait on a real
                    # Pool-stream instruction (1-wait-per-instruction ISA)
                    pn = nc.gpsimd.nop()
                    add_dep_helper(pn.ins, tps[-1].ins, sync=True,
                                   reason="pool observes xpose")
                cast = nc.gpsimd.dma_start(out=x_bf, in_=x_rtd[:, c * TPC:(c + 1) * TPC, :])
                if c > 0 or rep > 0:
                    add_dep_helper(cast.ins, pn.ins, sync=False, reason="pin")
                if rep > 0:
                    add_dep_helper(cast.ins, pnr.ins, sync=False, reason="pin")
                casts.append(cast)
                if casts[-2:-1]:
                    spn = nc.sync.nop()
                    add_dep_helper(spn.ins, tps[-1].ins, sync=True,
                                   reason="sp observes xpose")
                if rep > 0:
                    # xT readers of last rep: its last square (ACT) and mm (PE)
                    spn_a = nc.sync.nop()
                    add_dep_helper(spn_a.ins, prev_rep[c]["sq"].ins, sync=True,
                                   reason="rep: sp observes square")
                    spn_p = nc.sync.nop()
                    add_dep_helper(spn_p.ins, prev_rep[c]["mmB"].ins, sync=True,
                                   reason="rep: sp observes mm")
                tp = nc.sync.dma_start_transpose(
                    out=xTs[c][:],
                    in_=x_bf[:].rearrange("p t d -> p (t d)"))
                if casts[-2:-1]:
                    add_dep_helper(tp.ins, spn.ins, sync=False, reason="pin")
                if rep > 0:
                    add_dep_helper(tp.ins, spn_a.ins, sync=False, reason="pin")
                    add_dep_helper(tp.ins, spn_p.ins, sync=False, reason="pin")
                tps.append(tp)

                for bb in range(BPC):
                    b = c * BPC + bb
                    first = (rep == 0 and b == 0)
                    last = (rep == repeat - 1 and b == NB - 1)
                    if prev_mmB_last is not None:
                        i_touch = nc.scalar.copy(out=scr_b, in_=scr_a)
                        add_dep_helper(i_touch.ins, prev_mmB_last.ins, sync=True,
                                       reason="act observes PE")
                    xsq = xsqp.tile([128, TPB, 128], f16, tag="xsq")
                    i_sq = nc.scalar.square(
                        out=xsq, in_=xTs[c][:, bb * TPB:(bb + 1) * TPB, :])

                    psum_xc = pxc.tile([128, TPB * K], f32, tag="pxc")
                    mmB = None
                    for tt in range(TPB):
                        t = b * TPB + tt
                        nc.tensor.matmul(out=psum_xc[:, tt * K:(tt + 1) * K],
                                         lhsT=xTs[c][:, bb * TPB + tt, :], rhs=cTm2_sb,
                                         start=True, stop=False,
                                         skip_group_check=True)
                        mmB = nc.tensor.matmul(out=psum_xc[:, tt * K:(tt + 1) * K],
                                               lhsT=xsq[:, tt, :], rhs=ones64_sb,
                                               start=False, stop=True,
                                               skip_group_check=True)
                    prev_mmB_last = mmB

                    i_d2 = nc.vector.tensor_add(
                        out=dd[:, b * TPB:(b + 1) * TPB, 1, :],
                        in0=psum_xc[:].rearrange("p (t k) -> p t k", k=K),
                        in1=c2b_sb[:].rearrange("p (t k) -> p t k", k=K))
                    dscr = dscrp.tile([128, TPB, K], f16, tag="dscr")
                    i_sqrt = nc.scalar.activation(
                        out=dscr,
                        in_=dd[:, b * TPB:(b + 1) * TPB, 1, :],
                        func=mybir.ActivationFunctionType.Sqrt)
                    i_dc = nc.vector.tensor_copy(
                        out=dd[:, b * TPB:(b + 1) * TPB, 0, :], in_=dscr)

                    oh = ohp.tile([128, TPB, K], f16, tag="oh")
                    for tt in range(TPB):
                        t = b * TPB + tt
                        nc.vector.tensor_scalar(
                            out=oh[:, tt, :], in0=iota_sb,
                            scalar1=labf[:, t:t + 1], scalar2=None,
                            op0=mybir.AluOpType.is_equal)
                    # counts first: its DVE wait covers the G matmuls' oh dep
                    i_cnt = nc.tensor.matmul(
                        out=cnt_ps, lhsT=ones1,
                        rhs=oh[:].rearrange("p t k -> p (t k)"),
                        start=first, stop=last,
                        skip_group_check=True)
                    i_g = None
                    for tt in range(TPB):
                        t = b * TPB + tt
                        i_g = nc.tensor.matmul(
                            out=G_ps, lhsT=dd[:, t, :, :], rhs=oh[:, tt, :],
                            start=(first and tt == 0),
                            stop=(last and tt == TPB - 1),
                            skip_group_check=True)
                        if tt == 0:
                            add_dep_helper(i_g.ins, i_cnt.ins, sync=False,
                                           reason="cnt carries oh wait first")
                    if bb == BPC - 1:
                        prev_rep[c] = {"tp": tps[-1], "sq": i_sq, "mmB": prev_mmB_last}
                    if last:
                        end_list += [i_cnt, i_g, i_sqrt, i_d2]

            # ---- finale ----
            pn1 = nc.gpsimd.nop()
            add_dep_helper(pn1.ins, tps[-1].ins, sync=True, reason="pool observes xpose")
            pn2 = nc.gpsimd.nop()
            add_dep_helper(pn2.ins, casts[-1].ins, sync=True, reason="pool observes casts")
            G_sb = sb.tile([128, K], f32)
            i_gc = nc.vector.tensor_copy(out=G_sb, in_=G_ps)
            cnt_sb = sb.tile([1, TPB * K], f32)
            i_cc = nc.vector.tensor_copy(out=cnt_sb, in_=cnt_ps)
            e1 = nc.gpsimd.dma_start(out=out_G[:], in_=G_sb)
            add_dep_helper(e1.ins, pn1.ins, sync=False, reason="pin")
            add_dep_helper(e1.ins, pn2.ins, sync=False, reason="pin")
            pn3 = nc.gpsimd.nop()
            add_dep_helper(pn3.ins, e1.ins, sync=True, reason="pool observes e1")
            e2 = nc.gpsimd.dma_start(out=out_cnt[:], in_=cnt_sb)
            add_dep_helper(e2.ins, pn3.ins, sync=False, reason="pin")
            for inst in end_list + [i_labf, i_gc, i_cc, e1, e2, tps[-1], casts[-1]]:
                n = nc.sync.nop()
                add_dep_helper(n.ins, inst.ins, sync=True, reason="end chain")
    _strip_redundant_waits(nc)
    return nc


def _host_prep(centroids):
    c_bf = centroids.astype(ml_dtypes.bfloat16)
    c_f = c_bf.astype(np.float32)
    cTm2 = np.ascontiguousarray((-2.0 * c_f).T).astype(ml_dtypes.bfloat16)
    c2 = (c_f.astype(np.float64) ** 2).sum(1).astype(np.float32)
    c2b8 = np.broadcast_to(np.tile(c2, TPB), (128, TPB * K)).astype(np.float32).copy()
    iota = np.broadcast_to(np.arange(K, dtype=np.float32), (128, K)).copy()
    ones64 = np.ones((D, K), np.float16)
    return cTm2, c2b8, iota, ones64


def kernel(embeddings, cluster_labels, centroids):
    embeddings = np.ascontiguousarray(embeddings, dtype=np.float32)
    cluster_labels = np.ascontiguousarray(cluster_labels, dtype=np.int64)
    centroids = np.ascontiguousarray(centroids, dtype=np.float32)

    if "nc" not in _cache:
        _cache["nc"] = _build()
    nc = _cache["nc"]

    cTm2, c2b8, iota, ones64 = _host_prep(centroids)
    in_maps = []
    for cix in range(NCORES):
        xs = embeddings[cix * NS:(cix + 1) * NS]
        ls = cluster_labels[cix * NS:(cix + 1) * NS]
        in_maps.append({
            "x": xs,
            "lab": ls.view(np.int32).reshape(128, 2 * T),
            "cTm2": cTm2,
            "ones64": ones64,
            "iota64": iota,
            "c2b8": c2b8,
        })
    res = run_bass_kernel_spmd(nc, in_maps, core_ids=list(range(NCORES)))

    G = np.zeros((128, K), np.float64)
    cnt = np.zeros((TPB * K,), np.float64)
    for r in res.results:
        G += r["out_G"].astype(np.float64)
        cnt += r["out_cnt"].reshape(-1).astype(np.float64)
    counts = cnt.reshape(TPB, K).sum(0)                  # [K]
    G_dist = G[:K, :]
    G_d2 = G[K:, :]
    A = np.diag(G_d2)
    V = G_d2.sum(0) - A
    W = G_dist.sum(0) - np.diag(G_dist)
    safe = np.maximum(counts, 1.0)
    t_k = A + (MARGIN * MARGIN * (K - 1) * counts - 2.0 * MARGIN * W + V) / (K - 1)
    loss = np.where(counts > 0, t_k / safe, 0.0).sum() / K
    return np.float32(loss)

